# revision 2
# baseline (speedup 1.0000x reference)
"""Trainium2 Bass kernel for the masked note-accuracy loss.

Reference math (per sequence n):
    pred      = (sigmoid(x) > 0.5) = (x > 0)
    S_n       = sum_{t,d} pred * target                     (tru_pos)
    A[n,t]    = false_pos + false_neg = sum_d |pred - target|
    ratio     = S_n / (S_n + A[n,t]) = 2S_n / (2S_n + 2A[n,t])
    acc_n     = sum_{t<T_n} ratio / T_n,   T_n = sum_t mask[n,t]
    out       = sum_n acc_n
Sharding: data-parallel over N=128 sequences -> 16 per core on 8 cores;
the host sums the 8 per-core [128,16] partial tiles.

Per-core pipeline, one sequence per step ([T,D] loaded as a [128,16,88]
tile, t = p*16+k; x via the SP HWDGE queue, target via the ACT queue).
The mask is loaded ONCE in raw contiguous layout; T_i and the valid mask
are rebuilt on-chip from an iota-vs-T compare (== the reference's
(t < T_i) semantics):
  DVE p1: V = (x>0) - target (bf16), accum_out = per-partition (P-Q)
  DVE p2: -A[t] = negated segmented abs-reduce of V over d
  ACT   : Copy(2*target) with accum_out = per-partition 2Q
  mini epilogue (hidden under later sequences' DMAs): PE colsums ->
  one psum row [-A(16) | P-Q | 2Q]; its full reduce is directly 2S
  (sign trick); den = -2*(-A) + 2S; rat = recip(den)*2S*mask' with
  1/T_n folded into mask'; ratio colsums ACCUMULATE across sequences
  0..14 into one PSUM bank.

Tail shaping (the stream's last ~10 us): the exposed tail after the
final input byte is minimized by loading the LAST sequences in chunks:
  - seq 14 in four [P,4,D] quarters,
  - seq 15 in chunks [4,4,4,2,2] whose k=4 chunks are interleaved into
    the stream after seqs 8/10/12 (DVE absorbs them with its per-seq
    slack), and whose two k=2 chunks are the final bus transfers.
After the last 2-column chunk lands, only ~0.5us of DVE work plus the
epilogue chain and output DMA remain.  Chunk stats go to per-chunk
columns of stPQ/stQ2; p2 writes per-chunk column ranges of stA, so the
epilogue is uniform: colsum-matmul over [stA(16) | P-Q cols | 2Q cols].
Final: seq 15's rat tile gets the accumulated 0..14 row folded into
row 0 (via an ACT-staged SBUF copy of the PSUM row) and ships as one
[128,16] DMA; the host sums it.
"""

import numpy as np

import concourse.bacc as bacc
import concourse.tile as tile
from concourse import mybir
from concourse.alu_op_type import AluOpType
from concourse.bass_utils import run_bass_kernel_spmd

N, T, D = 128, 2048, 88
N_CORES = 8
NS = N // N_CORES
P = 128
K = T // P

_cached_nc = None

# chunk plan: CH[n] = list of (k0, kc) chunks for sequence n
CH = {n: [(0, K)] for n in range(NS)}
CH[14] = [(0, 4), (4, 4), (8, 4), (12, 4)]
CH[15] = [(0, 4), (4, 4), (8, 4), (12, 2), (14, 2)]
# seq-15 chunk i's load is issued right after load(SPREAD15[i]) for the
# first three; chunks 3 and 4 are issued after seq 14's quarters so they
# are the stream's final transfers.
SPREAD15 = [8, 10, 12]

# column base in stPQ/stQ2 for each sequence's chunks
COLB = {}
_c = 0
for _n in range(NS):
    COLB[_n] = _c
    _c += len(CH[_n])
NCOLS = _c


def _build():
    f32 = mybir.dt.float32
    vdt = mybir.dt.bfloat16
    nc = bacc.Bacc("TRN2", target_bir_lowering=False, debug=False,
                   num_devices=N_CORES)
    xd = nc.dram_tensor("output", [NS, T, D], f32, kind="ExternalInput")
    yd = nc.dram_tensor("target", [NS, T, D], f32, kind="ExternalInput")
    md = nc.dram_tensor("mask", [NS, T], mybir.dt.int32, kind="ExternalInput")
    od15 = nc.dram_tensor("partial15", [P, K], f32, kind="ExternalOutput")

    AX = mybir.AxisListType.X

    with tile.TileContext(nc) as tc:
        with (
            tc.tile_pool(name="data", bufs=3) as data_pool,
            tc.tile_pool(name="work", bufs=2) as work_pool,
            tc.tile_pool(name="mini", bufs=2) as mini_pool,
            tc.tile_pool(name="singles", bufs=1) as singles,
            tc.tile_pool(name="psl", bufs=2, space="PSUM") as psum_loop,
            tc.tile_pool(name="psk", bufs=1, space="PSUM") as psum_keep,
        ):
            stA = singles.tile([P, NS, K], f32)
            stPQ = singles.tile([P, NCOLS], f32)
            stQ2 = singles.tile([P, NCOLS], f32)
            maskf = singles.tile([P, NS, K], f32)
            mraw = singles.tile([P, T * NS // P], mybir.dt.int32)
            mrawf = singles.tile([P, T * NS // P], f32)
            rowsum = singles.tile([P, 1], f32)
            blockind = singles.tile([P, NS], f32)
            id16 = singles.tile([NS, NS], f32)
            t16 = singles.tile([NS, 1], f32)
            iota_t = singles.tile([P, K], mybir.dt.int32)
            ones128 = singles.tile([P, P], f32)
            inv_ti = singles.tile([1, NS], f32)
            row_ti = singles.tile([1, NS], f32)
            iota_pn = singles.tile([P, NS], mybir.dt.int32)
            iota_mn = singles.tile([NS, NS], mybir.dt.int32)
            tmp_pn = singles.tile([P, NS], f32)
            nc.vector.memset(ones128[:], 1.0)
            # blockind[p, n] = 1 iff p//8 == n, i.e. 0 <= p-8n < 8
            nc.gpsimd.iota(iota_pn[:], pattern=[[-8, NS]], base=0,
                           channel_multiplier=1)
            nc.vector.tensor_scalar(
                out=tmp_pn[:], in0=iota_pn[:], scalar1=0.0, scalar2=None,
                op0=AluOpType.is_ge)
            tmp_pn2 = singles.tile([P, NS], f32)
            nc.vector.tensor_scalar(
                out=tmp_pn2[:], in0=iota_pn[:], scalar1=8.0, scalar2=None,
                op0=AluOpType.is_lt)
            nc.vector.tensor_mul(blockind[:], tmp_pn[:], tmp_pn2[:])
            # id16[m, n] = (m == n)
            nc.gpsimd.iota(iota_mn[:], pattern=[[-1, NS]], base=0,
                           channel_multiplier=1)
            nc.vector.tensor_scalar(
                out=id16[:], in0=iota_mn[:], scalar1=0.0, scalar2=None,
                op0=AluOpType.is_equal)
            nc.gpsimd.iota(iota_t[:], pattern=[[1, K]], base=0,
                           channel_multiplier=K)
            ps_t16 = psum_keep.tile([NS, 1], f32)
            ps_ti = psum_keep.tile([1, NS], f32)
            ps_tb = psum_keep.tile([P, NS], f32)
            ps_itb = psum_keep.tile([P, NS], f32)
            ps_acc = psum_keep.tile([P, K], f32)
            sb_tb = singles.tile([P, NS], f32)
            sb_itb = singles.tile([P, NS], f32)
            sb_acc = singles.tile([1, K], f32)

            # dedicated tiles for the chunked tail loads (no pool WAR ->
            # descriptor gen runs early; program order fixes bus order)
            x14q = []
            y14q = []
            for ci, (k0, kc) in enumerate(CH[14]):
                xq = singles.tile([P, kc, D], f32, name=f"x14q{ci}")
                yq = singles.tile([P, kc, D], f32, name=f"y14q{ci}")
                x14q.append(xq)
                y14q.append(yq)
            x15c = []
            y15c = []
            for ci, (k0, kc) in enumerate(CH[15]):
                xq = singles.tile([P, kc, D], f32, name=f"x15c{ci}")
                yq = singles.tile([P, kc, D], f32, name=f"y15c{ci}")
                x15c.append(xq)
                y15c.append(yq)

            def load(n):
                xt = data_pool.tile([P, K, D], f32, tag="xt", name="xt")
                yt = data_pool.tile([P, K, D], f32, tag="yt", name="yt")
                nc.sync.dma_start(xt[:], xd.ap()[n].rearrange("(p k) d -> p k d", p=P))
                nc.scalar.dma_start(yt[:], yd.ap()[n].rearrange("(p k) d -> p k d", p=P))
                return xt, yt

            def load_chunk(n, ci, xtile, ytile):
                k0, kc = CH[n][ci]
                src = xd.ap()[n].rearrange("(p k) d -> p k d", p=P)[:, k0 : k0 + kc, :]
                nc.sync.dma_start(xtile[:], src)
                srcy = yd.ap()[n].rearrange("(p k) d -> p k d", p=P)[:, k0 : k0 + kc, :]
                nc.scalar.dma_start(ytile[:], srcy)

            def p1p2(n, ci, xt, yt):
                """per-chunk passes: p1 (V + P-Q accum), p2 (-A cols),
                ACT 2Q accum"""
                k0, kc = CH[n][ci]
                col = COLB[n] + ci
                v = work_pool.tile([P, kc, D], vdt, tag=f"v{kc}", name="v")
                nc.vector.scalar_tensor_tensor(
                    out=v[:], in0=xt[:], scalar=0.0, in1=yt[:],
                    op0=AluOpType.is_gt, op1=AluOpType.subtract,
                    accum_out=stPQ[:, col : col + 1],
                )
                nc.vector.tensor_reduce(
                    out=stA[:, n, k0 : k0 + kc], in_=v[:], axis=AX,
                    op=AluOpType.add, apply_absolute_value=True, negate=True,
                )
                scratch = work_pool.tile([P, kc, D], vdt, tag=f"s{kc}",
                                         name="scratch")
                nc.scalar.activation(
                    out=scratch[:], in_=yt[:],
                    func=mybir.ActivationFunctionType.Copy, scale=2.0,
                    accum_out=stQ2[:, col : col + 1],
                )

            def epilogue(n):
                ncn = len(CH[n])
                base = COLB[n]
                w = K + 2 * ncn
                ps_st = psum_loop.tile([P, K + 2 * 5], f32, tag="ps_st",
                                       name="ps_st")
                # stPQ/stQ2 colsums first (ready before the last p2)
                nc.tensor.matmul(ps_st[:, K : K + ncn], ones128[:],
                                 stPQ[:, base : base + ncn])
                nc.tensor.matmul(ps_st[:, K + ncn : K + 2 * ncn], ones128[:],
                                 stQ2[:, base : base + ncn])
                nc.tensor.matmul(ps_st[:, 0:K], ones128[:], stA[:, n, :])
                s2p = mini_pool.tile([P, 1], f32, tag="s2p", name="s2p")
                nc.vector.tensor_reduce(
                    out=s2p[:], in_=ps_st[:, 0:w], axis=AX, op=AluOpType.add)
                den = mini_pool.tile([P, K], f32, tag="den", name="den")
                nc.vector.tensor_scalar(
                    out=den[:], in0=stA[:, n, :],
                    scalar1=-2.0, scalar2=s2p[:], op0=AluOpType.mult,
                    op1=AluOpType.add)
                rec = mini_pool.tile([P, K], f32, tag="rec", name="rec")
                nc.vector.reciprocal(rec[:], den[:])
                rat = mini_pool.tile([P, K], f32, tag="rat", name="rat")
                nc.vector.scalar_tensor_tensor(
                    out=rat[:], in0=rec[:], scalar=s2p[:],
                    in1=maskf[:, n, :],
                    op0=AluOpType.mult, op1=AluOpType.mult)
                if n < NS - 1:
                    nc.tensor.matmul(ps_acc[:], ones128[:], rat[:],
                                     start=(n == 0), stop=(n == NS - 2))
                    if n == NS - 2:
                        # stage the accumulated row into SBUF via the idle
                        # scalar engine so seq 15's final fold is a
                        # pure-SBUF DVE add with no PSUM sync
                        nc.scalar.activation(
                            out=sb_acc[:], in_=ps_acc[0:1, :],
                            func=mybir.ActivationFunctionType.Copy)
                else:
                    nc.vector.tensor_tensor(
                        out=rat[0:1, :], in0=rat[0:1, :],
                        in1=sb_acc[:], op=AluOpType.add)
                    nc.sync.dma_start(od15.ap(), rat[:])

            xt0, yt0 = load(0)
            # raw contiguous mask load (1KB runs); T_i and the valid mask
            # are rebuilt on-chip: valid[t] = (t < T_i)
            nc.gpsimd.dma_start(
                mraw[:], md.ap().rearrange("n (g j) -> (n g) j", g=8))
            nc.vector.tensor_copy(mrawf[:], mraw[:])
            nc.vector.tensor_reduce(out=rowsum[:], in_=mrawf[:], axis=AX,
                                    op=AluOpType.add)
            nc.tensor.matmul(ps_t16[:], blockind[:], rowsum[:])
            nc.vector.tensor_copy(t16[:], ps_t16[:])
            nc.tensor.matmul(ps_ti[:], t16[:], id16[:])
            nc.vector.tensor_copy(row_ti[:], ps_ti[:])
            nc.vector.reciprocal(inv_ti[:], row_ti[:])
            nc.tensor.matmul(ps_tb[:], ones128[0:1, :], row_ti[:])
            nc.tensor.matmul(ps_itb[:], ones128[0:1, :], inv_ti[:])
            nc.vector.tensor_copy(sb_tb[:], ps_tb[:])
            nc.vector.tensor_copy(sb_itb[:], ps_itb[:])
            # maskf[p,n,k] = (t < T_n) / T_n
            for n in range(NS):
                nc.vector.tensor_scalar(
                    out=maskf[:, n, :], in0=iota_t[:],
                    scalar1=sb_tb[:, n : n + 1],
                    scalar2=sb_itb[:, n : n + 1], op0=AluOpType.is_lt,
                    op1=AluOpType.mult)

            # sequence 0
            p1p2(0, 0, xt0, yt0)
            epilogue(0)
            spread = {SPREAD15[i]: i for i in range(len(SPREAD15))}
            for n in range(1, 14):
                xt, yt = load(n)
                if n in spread:
                    ci = spread[n]
                    load_chunk(15, ci, x15c[ci], y15c[ci])
                p1p2(n, 0, xt, yt)
                if (n - 1) in spread:
                    ci = spread[n - 1]
                    p1p2(15, ci, x15c[ci], y15c[ci])
                epilogue(n)

            # seq 14 quarters, then the two final seq-15 chunks
            for ci in range(len(CH[14])):
                load_chunk(14, ci, x14q[ci], y14q[ci])
            load_chunk(15, 3, x15c[3], y15c[3])
            load_chunk(15, 4, x15c[4], y15c[4])

            p1p2(14, 0, x14q[0], y14q[0])
            p1p2(14, 1, x14q[1], y14q[1])
            p1p2(15, 2, x15c[2], y15c[2])
            p1p2(14, 2, x14q[2], y14q[2])
            p1p2(14, 3, x14q[3], y14q[3])
            epilogue(14)
            p1p2(15, 3, x15c[3], y15c[3])
            p1p2(15, 4, x15c[4], y15c[4])
            epilogue(15)

    nc.compile()
    return nc


def kernel(output, target, mask):
    global _cached_nc
    if _cached_nc is None:
        _cached_nc = _build()
    nc = _cached_nc
    output = np.asarray(output, dtype=np.float32)
    target = np.asarray(target, dtype=np.float32)
    mask = np.asarray(mask, dtype=np.int32)
    in_maps = []
    for c in range(N_CORES):
        sl = slice(c * NS, (c + 1) * NS)
        in_maps.append({
            "output": np.ascontiguousarray(output[sl]),
            "target": np.ascontiguousarray(target[sl]),
            "mask": np.ascontiguousarray(mask[sl]),
        })
    res = run_bass_kernel_spmd(nc, in_maps, list(range(N_CORES)))
    total = np.float32(0.0)
    for c in range(N_CORES):
        part = np.sum(res.results[c]["partial15"], dtype=np.float64)
        total = np.float32(total + np.float32(part))
    return np.float32(total)
